# revision 29
# baseline (speedup 1.0000x reference)
"""BinaryConv2D forward on 8 trn2 cores: flat-padded single-pass bf16 conv.

out = conv2d_same(inputs, sign(clip(kernel)))  (NHWC, HWIO, 3x3, stride 1)
Data-parallel over batch (4 images/core), kernel replicated (forward only).

Key design points (vs the 450us hi/lo 2-pass baseline):
  - Single bf16 pass: the correctness gate is 2e-2 max-rel; bf16-rounded
    inputs with exact +-1 weights give ~1.7e-3, so the fp32-emulating second
    (lo) pass is dropped -- halves the dominant PE matmul stream.
  - Flat-57 padded image: channel-major image stored flat with rows of
    57 = 56 pixels + ONE shared pad column (plus halo rows and 1-elem
    margins). A 3x3 tap shift (dy,dx) is then the constant flat offset
    dy*57+dx for every output position, so all 9 taps read contiguous
    [cin,128] windows of one tile.
  - Image-stationary matmuls: out[pix128, cout256] PSUM blocks accumulate
    18 matmuls (9 taps x 2 cin chunks); the output is born pixel-major and
    DMAs straight to HBM in row segments (no output transpose at all).
    25 blocks/image x 18 x 256 cols ~= 48us/image of PE stream.
  - Input path: NHWC row loads (SP/Act queues) -> GPSIMD bf16 cast -> PE
    transpose of [112pix,128cin] 2-row blocks -> DVE 3D-strided evict into
    the flat image. PE transposes interleave with the previous image's conv
    matmuls (~2.6us/image); input runs one image ahead via per-quarter
    injection between conv blocks, with one flat-image buffer per
    in-flight image.
  - Queue discipline: each pipeline stage owns a queue (loads SP/Act, casts
    Pool, evicts+PSUM-drain DVE, stores round-robin SP/Act/Pool) so a
    waiting input op never head-of-line-blocks the PSUM drain.
  - Startup: image-0 q0 loads go first; the weight stage arrives as 9
    per-tap DMA pieces (5 on the otherwise-idle Pool queue) and 18 per-tap
    signs race the first conv block, which the PE paces matmul-by-matmul.
    ~60 warmup matmuls cover the p-state ramp inside the dependency stalls.

Cost model: 213.4us vs 450.4us baseline (2.11x). PE busy ~203us = 97% of
span; conv stream 192us IS the floor for 128-position blocks (25 = minimum
window count covering 3136 outputs), and the transposes (10.5us) run at the
PE's 1 col/cycle transpose rate. Remaining span over PE-busy is the ~7.6us
startup head (weight DMA + per-tap signs racing the first conv block) and
the ~3.3us final-store pipeline latency, both latency-structural.
Loop-slope measurement on the 8 axon cores: 170.7us/iter in a clean
transport window (the steady-state loop body amortizes head and tail; the
axon transport itself drifts up to ~2x between windows, so cross-round
slope numbers vary). Correctness vs fp32 reference: max-rel 1.7e-3
(gate 2e-2).
"""

import numpy as np

P = 128
H = 56
W = 56
C = 256
XR = W + 1                   # padded row width: 56 pixels + 1 shared pad col
NCORES = 8
NTOT = 32
NI = NTOT // NCORES          # images per core
NPIX = H * W                 # 3136
NBLK = 28                    # 2-row blocks per image (input staging)
FLAT0 = XR                   # first output flat position (skip top halo row)
NFLAT = H * XR               # output flat positions incl pad cols (3192)
XQL = 1 + (H + 2) * XR + 1   # flat image length: margin + 58 rows + margin
MB = 128                     # output pixels (flat positions) per psum block
NWARM = 60                   # PE clock-ramp warmup matmuls

_cache = {}


def _store_segments():
    """Static (block, psum_range, hbm_pixel_range) store plan.

    Block b covers flat positions [FLAT0+128b, FLAT0+128b+M); pad columns
    (flat % 57 == 56) are garbage and split the stores into row segments.
    """
    plan = []
    f0g = FLAT0
    while f0g < FLAT0 + NFLAT:
        m = min(MB, FLAT0 + NFLAT - f0g)
        segs = []
        f = f0g
        while f < f0g + m:
            if f % XR == W:  # pad column
                f += 1
                continue
            e = min(f0g + m, (f // XR) * XR + W)  # run until pad col/block end
            r = f // XR - 1
            p = r * W + (f % XR)
            segs.append((f - f0g, e - f0g, p))
            f = e
        plan.append((f0g, m, segs))
        f0g += m
    return plan


def _build_bass(ni=NI, loops=1):
    import concourse.bacc as bacc
    import concourse.mybir as mybir
    import concourse.tile as tile
    from contextlib import ExitStack

    f32 = mybir.dt.float32
    bf16 = mybir.dt.bfloat16

    nc = bacc.Bacc()
    x = nc.dram_tensor("x", [ni, NPIX, C], f32, kind="ExternalInput")
    w = nc.dram_tensor("w", [3, 3, C, C], f32, kind="ExternalInput")
    y = nc.dram_tensor("y", [ni, NPIX, C], f32, kind="ExternalOutput")

    plan = _store_segments()

    with ExitStack() as ctx:
        tc = ctx.enter_context(tile.TileContext(nc))
        wpool = ctx.enter_context(tc.tile_pool(name="wpool", bufs=1))
        wstage = ctx.enter_context(tc.tile_pool(name="wstage", bufs=1))
        xqpool = ctx.enter_context(tc.tile_pool(name="xqpool", bufs=1))
        natp = ctx.enter_context(tc.tile_pool(name="natp", bufs=2))
        hinp = ctx.enter_context(tc.tile_pool(name="hinp", bufs=2))
        psc = ctx.enter_context(tc.tile_pool(name="psc", bufs=6, space="PSUM"))
        psb = ctx.enter_context(tc.tile_pool(name="psb", bufs=2, space="PSUM"))
        outp = ctx.enter_context(tc.tile_pool(name="outp", bufs=10))

        from concourse.masks import make_identity

        # HAM warmup: dummy matmuls keep the PE busy from t~0 so the activity
        # monitor lifts the clock gate before the first conv matmuls arrive.
        wst = wstage.tile([P, 9, 2, C], f32, name="wst")
        warm_ps = psc.tile([P, C], f32, name="ps")
        warmsrc = wpool.tile([P, P], bf16, name="warmsrc")
        nc.vector.memset(warmsrc, 0.0)
        identb = wpool.tile([P, P], bf16, name="identb")
        for _ in range(NWARM):
            nc.tensor.matmul(
                warm_ps[:, :P], lhsT=warmsrc, rhs=warmsrc, start=True, stop=True
            )
        make_identity(nc, identb)

        # flat padded images (bf16, channel-major), one buffer per in-flight
        # image (4: conv reads img i while input stages img i+1; loop-mode
        # also wraps img 0' during conv 3). Only the margins / halo rows /
        # pad columns are memset (they are all the conv's tap windows ever
        # read beyond the data the transposes write).
        xqs = []
        for i in range(4):
            t = xqpool.tile([P, 2, XQL], bf16, name=f"xq{i}")
            nc.vector.memset(t[:, :, 0:58], 0.0)
            nc.vector.memset(t[:, :, 1 + 57 * XR :], 0.0)
            padv = t[:, :, 2 * XR : 2 * XR + H * XR].rearrange(
                "p c (r s) -> p c r s", s=XR
            )[:, :, :, 0:1]
            nc.vector.memset(padv, 0.0)
            xqs.append(t)

        # ---- binarized weights: sign(w) as [cin, cout=256] bf16 slices.
        # Loaded as 9 per-tap pieces (Pool takes 5 since its queue is free
        # at t=0; SP/Act take 2 each after image 0's first-quarter loads),
        # with one sign per (tap, cc) so the first conv block's accumulation
        # stream can start as soon as the earliest taps are signed -- the PE
        # paces matmul-by-matmul against the racing signs. wsgn_emit() is
        # called by the builder after image 0's q0 loads are queued. ----
        wbig = wpool.tile([P, 9, 2, C], bf16, name="wbig")
        wsgn = {
            (ky, kx, cc): wbig[:, 3 * ky + kx, cc, :]
            for ky in range(3)
            for kx in range(3)
            for cc in range(2)
        }
        wsrc = w.rearrange("ky kx (cc p) o -> p (ky kx) cc o", p=P)
        piece_q = [nc.gpsimd, nc.gpsimd, nc.gpsimd, nc.gpsimd, nc.gpsimd,
                   nc.sync, nc.scalar, nc.sync, nc.scalar]

        def wsgn_emit():
            for j in range(9):
                piece_q[j].dma_start(
                    out=wst[:, j : j + 1], in_=wsrc[:, j : j + 1]
                )
            for j in range(9):
                for cc in range(2):
                    nc.scalar.sign(
                        out=wbig[:, j, cc, :], in_=wst[:, j, cc, :]
                    )

        store_engines = [nc.sync, nc.scalar, nc.gpsimd]

        def _alloc_input(img):
            tiles = {"xnat": [], "hin": [], "xrows": []}
            for cc in range(2):
                tiles["xnat"].append(natp.tile([112, NBLK, P], f32, name="xnat"))
                tiles["hin"].append(hinp.tile([112, NBLK, P], bf16, name="hin"))
                tiles["xrows"].append(
                    x[img, :, P * cc : P * (cc + 1)].rearrange(
                        "(b two xx) c -> xx b two c", two=2, xx=W
                    )
                )
            return tiles

        def _load_blocks(t, b0, b1):
            for cc in range(2):
                ldq = nc.sync if cc == 0 else nc.scalar
                for two in range(2):
                    ldq.dma_start(
                        out=t["xnat"][cc][W * two : W * (two + 1), b0:b1],
                        in_=t["xrows"][cc][:, b0:b1, two],
                    )
                nc.gpsimd.tensor_copy(
                    out=t["hin"][cc][:, b0:b1], in_=t["xnat"][cc][:, b0:b1]
                )

        def _input_q(img, q, t):
            # one input quarter: natural NHWC row loads + bf16 cast.
            _load_blocks(t, 7 * q, 7 * (q + 1))

        def _transp_range(img, rp0, rp1, t):
            # PE-transpose each 2-row block to channel-major and DVE-evict it
            # into the flat padded image (3D strided copy inserts the pad
            # column gap; pad cols/halo stay zero from the one-time memset).
            # On the PE these interleave with conv matmuls of the previous
            # image and never stall it; the DVE evicts depend only on
            # just-produced PSUM, so they cannot block the ob drains behind
            # them on the DVE queue.
            xq = xqs[img % 4]
            for cc in range(2):
                for rp in range(rp0, rp1):
                    pt = psb.tile([P, 112], bf16, name="ptb")
                    nc.tensor.transpose(
                        pt, t["hin"][cc][:, rp, :], identb[:112, :112]
                    )
                    base = 1 + (2 * rp + 1) * XR
                    dst = xq[:, cc, base : base + 2 * XR].rearrange(
                        "p (two c) -> p two c", two=2
                    )[:, :, :W]
                    nc.vector.tensor_copy(
                        out=dst,
                        in_=pt.rearrange("p (two c) -> p two c", two=2),
                    )

        def _transp_q(img, q, t):
            _transp_range(img, 7 * q, 7 * (q + 1), t)

        def _conv_block(img, bi):
            xq = xqs[img % 4]
            f0, m, segs = plan[bi]
            ps = psc.tile([P, C], f32, name="ps")
            ci = 0
            for cc in range(2):
                for ky in range(3):
                    for kx in range(3):
                        s = (ky - 1) * XR + (kx - 1)
                        nc.tensor.matmul(
                            ps[:m, :],
                            lhsT=xq[:, cc, 1 + f0 + s : 1 + f0 + s + m],
                            rhs=wsgn[(ky, kx, cc)],
                            start=(ci == 0),
                            stop=(ci == 17),
                        )
                        ci += 1
            ob = outp.tile([P, C], f32, name="ob")
            nc.vector.tensor_copy(out=ob[:m, :], in_=ps[:m, :])
            for si, (a, e, p0) in enumerate(segs):
                eng = store_engines[(bi + si) % len(store_engines)]
                eng.dma_start(
                    out=y[img, p0 : p0 + (e - a), :], in_=ob[a:e, :]
                )

        INJ_LOAD = (1, 5, 9, 13)   # conv blocks at which the next image's
        INJ_TR = (4, 8, 12, 16)    # quarter loads / PE-transposes are issued

        def _images():
            t = _alloc_input(0)
            _input_q(0, 0, t)
            wsgn_emit()
            _transp_q(0, 0, t)
            for q in range(1, 4):
                _input_q(0, q, t)
                _transp_q(0, q, t)
            for img in range(ni):
                nxt = _alloc_input(img + 1) if img + 1 < ni else None
                for bi in range(len(plan)):
                    if nxt is not None and bi in INJ_LOAD:
                        _input_q(img + 1, INJ_LOAD.index(bi), nxt)
                    if nxt is not None and bi in INJ_TR:
                        _transp_q(img + 1, INJ_TR.index(bi), nxt)
                    _conv_block(img, bi)

        if loops == 1:
            _images()
        else:
            # loop-mode (timing builds): steady-state body with
            # cross-iteration input prefetch -- image (img+1)%ni is staged
            # during conv(img), so iteration boundaries pay no pipeline head.
            # The preamble primes image 0 once; with 4 xq buffers the wrap
            # (conv(3) reading buf 3 while input(0') writes buf 0) is safe.
            t0 = _alloc_input(0)
            _input_q(0, 0, t0)
            wsgn_emit()
            _transp_q(0, 0, t0)
            for q in range(1, 4):
                _input_q(0, q, t0)
                _transp_q(0, q, t0)
            with tc.For_i(0, loops, 1):
                for img in range(ni):
                    nxt_img = (img + 1) % ni
                    nxt = _alloc_input(nxt_img)
                    for bi in range(len(plan)):
                        if bi in INJ_LOAD:
                            _input_q(nxt_img, INJ_LOAD.index(bi), nxt)
                        if bi in INJ_TR:
                            _transp_q(nxt_img, INJ_TR.index(bi), nxt)
                        _conv_block(img, bi)
    nc.compile()
    return nc


def get_bass(ni=NI, loops=1):
    key = (ni, loops)
    if key not in _cache:
        _cache[key] = _build_bass(ni, loops)
    return _cache[key]


def run(inputs, kernel, trace=False, **kw):
    from concourse.bass_utils import run_bass_kernel_spmd

    nc = get_bass()
    xs = np.ascontiguousarray(inputs, dtype=np.float32).reshape(NTOT, NPIX, C)
    wf = np.ascontiguousarray(kernel, dtype=np.float32)
    in_maps = [
        {"x": xs[i * NI : (i + 1) * NI], "w": wf} for i in range(NCORES)
    ]
    res = run_bass_kernel_spmd(nc, in_maps, core_ids=list(range(NCORES)),
                               trace=trace, **kw)
    out = np.concatenate([r["y"] for r in res.results], axis=0)
    return out.reshape(NTOT, H, W, C), res


def kernel(**inputs):
    out, _ = run(inputs["inputs"], inputs["kernel"])
    return out


# revision 30
# speedup vs baseline: 1.6155x; 1.6155x over previous
"""BinaryConv2D forward on 8 trn2 cores: flat-padded single-pass bf16 conv.

out = conv2d_same(inputs, sign(clip(kernel)))  (NHWC, HWIO, 3x3, stride 1)
Data-parallel over batch (4 images/core), kernel replicated (forward only).

Key design points (vs the 450us hi/lo 2-pass baseline):
  - Single bf16 pass: the correctness gate is 2e-2 max-rel; bf16-rounded
    inputs with exact +-1 weights give ~1.7e-3, so the fp32-emulating second
    (lo) pass is dropped -- halves the dominant PE matmul stream.
  - Flat-57 padded image: channel-major image stored flat with rows of
    57 = 56 pixels + ONE shared pad column (plus halo rows and 1-elem
    margins). A 3x3 tap shift (dy,dx) is then the constant flat offset
    dy*57+dx for every output position, so all 9 taps read contiguous
    [cin,128] windows of one tile.
  - Image-stationary matmuls: out[pix128, cout256] PSUM blocks accumulate
    18 matmuls (9 taps x 2 cin chunks); the output is born pixel-major and
    DMAs straight to HBM in row segments (no output transpose at all).
    25 blocks/image x 18 x 256 cols ~= 48us/image of PE stream.
  - Input path: NHWC row loads (SP/Act queues) -> GPSIMD bf16 cast -> PE
    transpose of [112pix,128cin] 2-row blocks -> DVE 3D-strided evict into
    the flat image. PE transposes interleave with the previous image's conv
    matmuls (~2.6us/image); input runs one image ahead via per-quarter
    injection between conv blocks, with one flat-image buffer per
    in-flight image.
  - Queue discipline: each pipeline stage owns a queue (loads SP/Act, casts
    Pool, evicts+PSUM-drain DVE, stores round-robin SP/Act/Pool) so a
    waiting input op never head-of-line-blocks the PSUM drain.
  - Startup: image-0 q0 loads go first; the weight stage arrives as 9
    per-tap DMA pieces (5 on the otherwise-idle Pool queue) and 18 per-tap
    signs race the first conv block, which the PE paces matmul-by-matmul.
    ~60 warmup matmuls cover the p-state ramp inside the dependency stalls.

Cost model: 213.4us vs 450.4us baseline (2.11x). PE busy ~203us = 97% of
span; conv stream 192us IS the floor for 128-position blocks (25 = minimum
window count covering 3136 outputs), and the transposes (10.5us) run at the
PE's 1 col/cycle transpose rate. Remaining span over PE-busy is the ~7.6us
startup head (weight DMA + per-tap signs racing the first conv block) and
the ~3.3us final-store pipeline latency, both latency-structural.
Loop-slope measurement on the 8 axon cores: 170.7us/iter in a clean
transport window (the steady-state loop body amortizes head and tail; the
axon transport itself drifts up to ~2x between windows, so cross-round
slope numbers vary). Correctness vs fp32 reference: max-rel 1.7e-3
(gate 2e-2).
"""

import numpy as np

P = 128
H = 56
W = 56
C = 256
XR = W + 1                   # padded row width: 56 pixels + 1 shared pad col
NCORES = 8
NTOT = 32
NI = NTOT // NCORES          # images per core
NPIX = H * W                 # 3136
NBLK = 28                    # 2-row blocks per image (input staging)
FLAT0 = XR                   # first output flat position (skip top halo row)
NFLAT = H * XR               # output flat positions incl pad cols (3192)
XQL = 1 + (H + 2) * XR + 1   # flat image length: margin + 58 rows + margin
XQS = 3312                   # xq plane stride: XQL padded to 16B alignment
                             # (dual-fp8 DoubleRow Ldweights requires outer
                             # free-AP steps even and 16-aligned)
MB = 128                     # output pixels (flat positions) per psum block
NWARM = 60                   # PE clock-ramp warmup matmuls

_cache = {}


def _store_segments():
    """Static (block, psum_range, hbm_pixel_range) store plan.

    Block b covers flat positions [FLAT0+128b, FLAT0+128b+M); pad columns
    (flat % 57 == 56) are garbage and split the stores into row segments.
    """
    plan = []
    f0g = FLAT0
    while f0g < FLAT0 + NFLAT:
        m = min(MB, FLAT0 + NFLAT - f0g)
        segs = []
        f = f0g
        while f < f0g + m:
            if f % XR == W:  # pad column
                f += 1
                continue
            e = min(f0g + m, (f // XR) * XR + W)  # run until pad col/block end
            r = f // XR - 1
            p = r * W + (f % XR)
            segs.append((f - f0g, e - f0g, p))
            f = e
        plan.append((f0g, m, segs))
        f0g += m
    return plan


def _build_bass(ni=NI, loops=1):
    import concourse.bacc as bacc
    import concourse.mybir as mybir
    import concourse.tile as tile
    from contextlib import ExitStack

    f32 = mybir.dt.float32
    bf16 = mybir.dt.bfloat16

    nc = bacc.Bacc()
    x = nc.dram_tensor("x", [ni, NPIX, C], f32, kind="ExternalInput")
    w = nc.dram_tensor("w", [3, 3, C, C], f32, kind="ExternalInput")
    y = nc.dram_tensor("y", [ni, NPIX, C], f32, kind="ExternalOutput")

    plan = _store_segments()

    with ExitStack() as ctx:
        tc = ctx.enter_context(tile.TileContext(nc))
        wpool = ctx.enter_context(tc.tile_pool(name="wpool", bufs=1))
        wstage = ctx.enter_context(tc.tile_pool(name="wstage", bufs=1))
        xqpool = ctx.enter_context(tc.tile_pool(name="xqpool", bufs=1))
        natp = ctx.enter_context(tc.tile_pool(name="natp", bufs=4))
        hinp = ctx.enter_context(tc.tile_pool(name="hinp", bufs=4))
        psc = ctx.enter_context(tc.tile_pool(name="psc", bufs=6, space="PSUM"))
        psb = ctx.enter_context(tc.tile_pool(name="psb", bufs=2, space="PSUM"))
        outp = ctx.enter_context(tc.tile_pool(name="outp", bufs=10))

        from concourse.masks import make_identity

        # HAM warmup: dummy matmuls keep the PE busy from t~0 so the activity
        # monitor lifts the clock gate before the first conv matmuls arrive.
        wst = wstage.tile([P, 9, 2, C], f32, name="wst")
        warm_ps = psc.tile([P, C], f32, name="ps")
        warmsrc = wpool.tile([P, P], bf16, name="warmsrc")
        nc.vector.memset(warmsrc, 0.0)
        identb = wpool.tile([P, P], bf16, name="identb")
        for _ in range(NWARM):
            nc.tensor.matmul(
                warm_ps[:, :P], lhsT=warmsrc, rhs=warmsrc, start=True, stop=True
            )
        make_identity(nc, identb)

        # flat padded images (bf16, channel-major), one buffer per in-flight
        # image (4: conv reads img i while input stages img i+1; loop-mode
        # also wraps img 0' during conv 3). Only the margins / halo rows /
        # pad columns are memset (they are all the conv's tap windows ever
        # read beyond the data the transposes write).
        f8 = mybir.dt.float8e4
        xqs = []
        for i in range(4):
            pair = []
            for tag in ("h", "l"):
                t = xqpool.tile([P, 2, XQS], f8, name=f"xq{tag}{i}")
                nc.vector.memset(t[:, :, 0:58], 0.0)
                nc.vector.memset(t[:, :, 1 + 57 * XR :], 0.0)
                padv = t[:, :, 2 * XR : 2 * XR + H * XR].rearrange(
                    "p c (r s) -> p c r s", s=XR
                )[:, :, :, 0:1]
                nc.vector.memset(padv, 0.0)
                pair.append(t)
            xqs.append(pair)

        # ---- binarized weights: sign(w) as [cin, cout=256] bf16 slices.
        # Loaded as 9 per-tap pieces (Pool takes 5 since its queue is free
        # at t=0; SP/Act take 2 each after image 0's first-quarter loads),
        # with one sign per (tap, cc) so the first conv block's accumulation
        # stream can start as soon as the earliest taps are signed -- the PE
        # paces matmul-by-matmul against the racing signs. wsgn_emit() is
        # called by the builder after image 0's q0 loads are queued. ----
        wbig = wpool.tile([P, 9, 2, C], f8, name="wbig")
        wsgn = {
            (ky, kx, cc): wbig[:, 3 * ky + kx, cc, :]
            for ky in range(3)
            for kx in range(3)
            for cc in range(2)
        }
        wsrc = w.rearrange("ky kx (cc p) o -> p (ky kx) cc o", p=P)
        piece_q = [nc.gpsimd, nc.gpsimd, nc.gpsimd, nc.gpsimd, nc.gpsimd,
                   nc.sync, nc.scalar, nc.sync, nc.scalar]

        def wsgn_emit():
            for j in range(9):
                piece_q[j].dma_start(
                    out=wst[:, j : j + 1], in_=wsrc[:, j : j + 1]
                )
            for j in range(9):
                for cc in range(2):
                    nc.scalar.sign(
                        out=wbig[:, j, cc, :], in_=wst[:, j, cc, :]
                    )

        store_engines = [nc.sync, nc.scalar, nc.gpsimd]

        def _alloc_input(img):
            tiles = {"xnat": [], "hin": [], "xrows": []}
            for cc in range(2):
                tiles["xnat"].append(natp.tile([112, NBLK, P], f32, name="xnat"))
                tiles["hin"].append(hinp.tile([112, NBLK, P], bf16, name="hin"))
                tiles["xrows"].append(
                    x[img, :, P * cc : P * (cc + 1)].rearrange(
                        "(b two xx) c -> xx b two c", two=2, xx=W
                    )
                )
            return tiles

        def _load_blocks(t, b0, b1):
            for cc in range(2):
                ldq = nc.sync if cc == 0 else nc.scalar
                for two in range(2):
                    ldq.dma_start(
                        out=t["xnat"][cc][W * two : W * (two + 1), b0:b1],
                        in_=t["xrows"][cc][:, b0:b1, two],
                    )
                nc.gpsimd.tensor_copy(
                    out=t["hin"][cc][:, b0:b1], in_=t["xnat"][cc][:, b0:b1]
                )

        def _input_q(img, q, t):
            # one input quarter: natural NHWC row loads + bf16 cast.
            _load_blocks(t, 7 * q, 7 * (q + 1))

        def _transp_range(img, rp0, rp1, t):
            # PE-transpose each 2-row block to channel-major and DVE-evict it
            # into the flat padded image (3D strided copy inserts the pad
            # column gap; pad cols/halo stay zero from the one-time memset).
            # On the PE these interleave with conv matmuls of the previous
            # image and never stall it; the DVE evicts depend only on
            # just-produced PSUM, so they cannot block the ob drains behind
            # them on the DVE queue.
            xqh, xql = xqs[img % 4]
            nrp = rp1 - rp0
            for cc in range(2):
                ptq = psb.tile([P, 7, 112], bf16, name="ptb")
                for rpi in range(nrp):
                    nc.tensor.transpose(
                        ptq[:, rpi, :],
                        t["hin"][cc][:, rp0 + rpi, :],
                        identb[:112, :112],
                    )
                # one coarse hi-copy + lo-subtract per (quarter, cc): the
                # 448 fine-grained evicts paid ~0.15us DVE overhead each
                base = 1 + (2 * rp0 + 1) * XR
                ptv = ptq[:, :nrp, :].rearrange(
                    "p rp (two c) -> p rp two c", two=2
                )
                span = 2 * nrp * XR
                dsth = xqh[:, cc, base : base + span].rearrange(
                    "p (rp two c) -> p rp two c", rp=nrp, two=2
                )[:, :, :, :W]
                dstl = xql[:, cc, base : base + span].rearrange(
                    "p (rp two c) -> p rp two c", rp=nrp, two=2
                )[:, :, :, :W]
                nc.vector.tensor_copy(out=dsth, in_=ptv)
                nc.vector.tensor_tensor(
                    out=dstl, in0=ptv, in1=dsth,
                    op=mybir.AluOpType.subtract,
                )

        def _transp_q(img, q, t):
            _transp_range(img, 7 * q, 7 * (q + 1), t)

        def _conv_block(img, bi):
            xqh, xql = xqs[img % 4]
            f0, m, segs = plan[bi]
            ps = psc.tile([P, C], f32, name="ps")
            ci = 0
            # fp8 DoubleRow: one matmul per (tap, hi/lo) contracts BOTH cin
            # chunks -- lhsT [128, 2cc, M] / rhs [128, 2cc, 256] -> [M, 256].
            # x = hi + lo with hi = fp8(x), lo = fp8(x - hi); weights are
            # +-1, exact in fp8e4.
            for src_img in (xqh, xql):
                for ky in range(3):
                    for kx in range(3):
                        s = (ky - 1) * XR + (kx - 1)
                        nc.tensor.matmul(
                            ps[:m, :],
                            lhsT=src_img[:, :, 1 + f0 + s : 1 + f0 + s + m],
                            rhs=wbig[:, 3 * ky + kx],
                            start=(ci == 0),
                            stop=(ci == 17),
                            perf_mode=mybir.MatmulPerfMode.DoubleRow,
                        )
                        ci += 1
            ob = outp.tile([P, C], f32, name="ob")
            if bi % 2 == 0:
                nc.vector.tensor_copy(out=ob[:m, :], in_=ps[:m, :])
            else:
                nc.scalar.copy(out=ob[:m, :], in_=ps[:m, :])
            for si, (a, e, p0) in enumerate(segs):
                eng = store_engines[(bi + si) % len(store_engines)]
                eng.dma_start(
                    out=y[img, p0 : p0 + (e - a), :], in_=ob[a:e, :]
                )

        INJ_LOAD = (6, 12, 17, 21)  # conv blocks at which the next image's
        INJ_TR = (8, 14, 19, 23)    # quarter loads / PE-transposes are issued
        OWN_Q = {4: 1, 11: 2, 17: 3}  # image 0 stages its own quarter q
        # just before the first conv block that needs those input rows

        def _images():
            t = _alloc_input(0)
            _input_q(0, 0, t)
            wsgn_emit()
            _transp_q(0, 0, t)
            for img in range(ni):
                nxt = _alloc_input(img + 1) if img + 1 < ni else None
                for bi in range(len(plan)):
                    if img == 0 and bi in OWN_Q:
                        _input_q(0, OWN_Q[bi], t)
                        _transp_q(0, OWN_Q[bi], t)
                    if nxt is not None and bi in INJ_LOAD:
                        _input_q(img + 1, INJ_LOAD.index(bi), nxt)
                    if nxt is not None and bi in INJ_TR:
                        _transp_q(img + 1, INJ_TR.index(bi), nxt)
                    _conv_block(img, bi)

        if loops == 1:
            _images()
        else:
            # loop-mode (timing builds): steady-state body with
            # cross-iteration input prefetch -- image (img+1)%ni is staged
            # during conv(img), so iteration boundaries pay no pipeline head.
            # The preamble primes image 0 once; with 4 xq buffers the wrap
            # (conv(3) reading buf 3 while input(0') writes buf 0) is safe.
            t0 = _alloc_input(0)
            _input_q(0, 0, t0)
            wsgn_emit()
            _transp_q(0, 0, t0)
            for q in range(1, 4):
                _input_q(0, q, t0)
                _transp_q(0, q, t0)
            with tc.For_i(0, loops, 1):
                for img in range(ni):
                    nxt_img = (img + 1) % ni
                    nxt = _alloc_input(nxt_img)
                    for bi in range(len(plan)):
                        if bi in INJ_LOAD:
                            _input_q(nxt_img, INJ_LOAD.index(bi), nxt)
                        if bi in INJ_TR:
                            _transp_q(nxt_img, INJ_TR.index(bi), nxt)
                        _conv_block(img, bi)
    nc.compile()
    return nc


def get_bass(ni=NI, loops=1):
    key = (ni, loops)
    if key not in _cache:
        _cache[key] = _build_bass(ni, loops)
    return _cache[key]


def run(inputs, kernel, trace=False, **kw):
    from concourse.bass_utils import run_bass_kernel_spmd

    nc = get_bass()
    xs = np.ascontiguousarray(inputs, dtype=np.float32).reshape(NTOT, NPIX, C)
    wf = np.ascontiguousarray(kernel, dtype=np.float32)
    in_maps = [
        {"x": xs[i * NI : (i + 1) * NI], "w": wf} for i in range(NCORES)
    ]
    res = run_bass_kernel_spmd(nc, in_maps, core_ids=list(range(NCORES)),
                               trace=trace, **kw)
    out = np.concatenate([r["y"] for r in res.results], axis=0)
    return out.reshape(NTOT, H, W, C), res


def kernel(**inputs):
    out, _ = run(inputs["inputs"], inputs["kernel"])
    return out


# revision 31
# speedup vs baseline: 1.6514x; 1.0223x over previous
"""BinaryConv2D forward on 8 trn2 cores: flat-padded single-pass bf16 conv.

out = conv2d_same(inputs, sign(clip(kernel)))  (NHWC, HWIO, 3x3, stride 1)
Data-parallel over batch (4 images/core), kernel replicated (forward only).

Key design points (vs the 450us hi/lo 2-pass baseline):
  - Single bf16 pass: the correctness gate is 2e-2 max-rel; bf16-rounded
    inputs with exact +-1 weights give ~1.7e-3, so the fp32-emulating second
    (lo) pass is dropped -- halves the dominant PE matmul stream.
  - Flat-57 padded image: channel-major image stored flat with rows of
    57 = 56 pixels + ONE shared pad column (plus halo rows and 1-elem
    margins). A 3x3 tap shift (dy,dx) is then the constant flat offset
    dy*57+dx for every output position, so all 9 taps read contiguous
    [cin,128] windows of one tile.
  - Image-stationary matmuls: out[pix128, cout256] PSUM blocks accumulate
    18 matmuls (9 taps x 2 cin chunks); the output is born pixel-major and
    DMAs straight to HBM in row segments (no output transpose at all).
    25 blocks/image x 18 x 256 cols ~= 48us/image of PE stream.
  - Input path: NHWC row loads (SP/Act queues) -> GPSIMD bf16 cast -> PE
    transpose of [112pix,128cin] 2-row blocks -> DVE 3D-strided evict into
    the flat image. PE transposes interleave with the previous image's conv
    matmuls (~2.6us/image); input runs one image ahead via per-quarter
    injection between conv blocks, with one flat-image buffer per
    in-flight image.
  - Queue discipline: each pipeline stage owns a queue (loads SP/Act, casts
    Pool, evicts+PSUM-drain DVE, stores round-robin SP/Act/Pool) so a
    waiting input op never head-of-line-blocks the PSUM drain.
  - Startup: image-0 q0 loads go first; the weight stage arrives as 9
    per-tap DMA pieces (5 on the otherwise-idle Pool queue) and 18 per-tap
    signs race the first conv block, which the PE paces matmul-by-matmul.
    ~60 warmup matmuls cover the p-state ramp inside the dependency stalls.

Cost model: 213.4us vs 450.4us baseline (2.11x). PE busy ~203us = 97% of
span; conv stream 192us IS the floor for 128-position blocks (25 = minimum
window count covering 3136 outputs), and the transposes (10.5us) run at the
PE's 1 col/cycle transpose rate. Remaining span over PE-busy is the ~7.6us
startup head (weight DMA + per-tap signs racing the first conv block) and
the ~3.3us final-store pipeline latency, both latency-structural.
Loop-slope measurement on the 8 axon cores: 170.7us/iter in a clean
transport window (the steady-state loop body amortizes head and tail; the
axon transport itself drifts up to ~2x between windows, so cross-round
slope numbers vary). Correctness vs fp32 reference: max-rel 1.7e-3
(gate 2e-2).
"""

import numpy as np

P = 128
H = 56
W = 56
C = 256
XR = W + 1                   # padded row width: 56 pixels + 1 shared pad col
NCORES = 8
NTOT = 32
NI = NTOT // NCORES          # images per core
NPIX = H * W                 # 3136
NBLK = 28                    # 2-row blocks per image (input staging)
FLAT0 = XR                   # first output flat position (skip top halo row)
NFLAT = H * XR               # output flat positions incl pad cols (3192)
XQL = 1 + (H + 2) * XR + 1   # flat image length: margin + 58 rows + margin
XQS = 3312                   # xq plane stride: XQL padded to 16B alignment
                             # (dual-fp8 DoubleRow Ldweights requires outer
                             # free-AP steps even and 16-aligned)
MB = 128                     # output pixels (flat positions) per psum block
NWARM = 60                   # PE clock-ramp warmup matmuls

_cache = {}


def _store_segments():
    """Static (block, psum_range, hbm_pixel_range) store plan.

    Block b covers flat positions [FLAT0+128b, FLAT0+128b+M); pad columns
    (flat % 57 == 56) are garbage and split the stores into row segments.
    """
    plan = []
    f0g = FLAT0
    while f0g < FLAT0 + NFLAT:
        m = min(MB, FLAT0 + NFLAT - f0g)
        segs = []
        f = f0g
        while f < f0g + m:
            if f % XR == W:  # pad column
                f += 1
                continue
            e = min(f0g + m, (f // XR) * XR + W)  # run until pad col/block end
            r = f // XR - 1
            p = r * W + (f % XR)
            segs.append((f - f0g, e - f0g, p))
            f = e
        plan.append((f0g, m, segs))
        f0g += m
    return plan


def _build_bass(ni=NI, loops=1):
    import concourse.bacc as bacc
    import concourse.mybir as mybir
    import concourse.tile as tile
    from contextlib import ExitStack

    f32 = mybir.dt.float32
    bf16 = mybir.dt.bfloat16

    nc = bacc.Bacc()
    x = nc.dram_tensor("x", [ni, NPIX, C], f32, kind="ExternalInput")
    w = nc.dram_tensor("w", [3, 3, C, C], f32, kind="ExternalInput")
    y = nc.dram_tensor("y", [ni, NPIX, C], f32, kind="ExternalOutput")

    plan = _store_segments()

    with ExitStack() as ctx:
        tc = ctx.enter_context(tile.TileContext(nc))
        wpool = ctx.enter_context(tc.tile_pool(name="wpool", bufs=1))
        wstage = ctx.enter_context(tc.tile_pool(name="wstage", bufs=1))
        xqpool = ctx.enter_context(tc.tile_pool(name="xqpool", bufs=1))
        natp = ctx.enter_context(tc.tile_pool(name="natp", bufs=4))
        hinp = ctx.enter_context(tc.tile_pool(name="hinp", bufs=4))
        psc = ctx.enter_context(tc.tile_pool(name="psc", bufs=6, space="PSUM"))
        psb = ctx.enter_context(tc.tile_pool(name="psb", bufs=2, space="PSUM"))
        outp = ctx.enter_context(tc.tile_pool(name="outp", bufs=10))

        from concourse.masks import make_identity

        # HAM warmup: dummy matmuls keep the PE busy from t~0 so the activity
        # monitor lifts the clock gate before the first conv matmuls arrive.
        wst = wstage.tile([P, 9, 2, C], f32, name="wst")
        warm_ps = psc.tile([P, C], f32, name="ps")
        warmsrc = wpool.tile([P, P], bf16, name="warmsrc")
        nc.vector.memset(warmsrc, 0.0)
        identb = wpool.tile([P, P], bf16, name="identb")
        for _ in range(NWARM):
            nc.tensor.matmul(
                warm_ps[:, :P], lhsT=warmsrc, rhs=warmsrc, start=True, stop=True
            )
        make_identity(nc, identb)

        # flat padded images (bf16, channel-major), one buffer per in-flight
        # image (4: conv reads img i while input stages img i+1; loop-mode
        # also wraps img 0' during conv 3). Only the margins / halo rows /
        # pad columns are memset (they are all the conv's tap windows ever
        # read beyond the data the transposes write).
        f8 = mybir.dt.float8e4
        xqs = []
        for i in range(4):
            pair = []
            for tag in ("h", "l"):
                t = xqpool.tile([P, 2, XQS], f8, name=f"xq{tag}{i}")
                nc.vector.memset(t[:, :, 0:58], 0.0)
                nc.vector.memset(t[:, :, 1 + 57 * XR :], 0.0)
                padv = t[:, :, 2 * XR : 2 * XR + H * XR].rearrange(
                    "p c (r s) -> p c r s", s=XR
                )[:, :, :, 0:1]
                nc.vector.memset(padv, 0.0)
                pair.append(t)
            xqs.append(pair)

        # ---- binarized weights: sign(w) as [cin, cout=256] bf16 slices.
        # Loaded as 9 per-tap pieces (Pool takes 5 since its queue is free
        # at t=0; SP/Act take 2 each after image 0's first-quarter loads),
        # with one sign per (tap, cc) so the first conv block's accumulation
        # stream can start as soon as the earliest taps are signed -- the PE
        # paces matmul-by-matmul against the racing signs. wsgn_emit() is
        # called by the builder after image 0's q0 loads are queued. ----
        wbig = wpool.tile([P, 9, 2, C], f8, name="wbig")
        wsgn = {
            (ky, kx, cc): wbig[:, 3 * ky + kx, cc, :]
            for ky in range(3)
            for kx in range(3)
            for cc in range(2)
        }
        wsrc = w.rearrange("ky kx (cc p) o -> p (ky kx) cc o", p=P)
        piece_q = [nc.gpsimd, nc.gpsimd, nc.gpsimd, nc.gpsimd, nc.gpsimd,
                   nc.sync, nc.scalar, nc.sync, nc.scalar]

        def wsgn_emit():
            for j in range(9):
                piece_q[j].dma_start(
                    out=wst[:, j : j + 1], in_=wsrc[:, j : j + 1]
                )
            for j in range(9):
                for cc in range(2):
                    nc.scalar.sign(
                        out=wbig[:, j, cc, :], in_=wst[:, j, cc, :]
                    )

        store_engines = [nc.sync, nc.scalar, nc.gpsimd]

        def _alloc_input(img):
            tiles = {"xnat": [], "hin": [], "xrows": []}
            for cc in range(2):
                tiles["xnat"].append(natp.tile([112, NBLK, P], f32, name="xnat"))
                tiles["hin"].append(hinp.tile([112, NBLK, P], bf16, name="hin"))
                tiles["xrows"].append(
                    x[img, :, P * cc : P * (cc + 1)].rearrange(
                        "(b two xx) c -> xx b two c", two=2, xx=W
                    )
                )
            return tiles

        def _load_blocks(t, b0, b1):
            for cc in range(2):
                ldq = nc.sync if cc == 0 else nc.scalar
                for two in range(2):
                    ldq.dma_start(
                        out=t["xnat"][cc][W * two : W * (two + 1), b0:b1],
                        in_=t["xrows"][cc][:, b0:b1, two],
                    )
                nc.gpsimd.tensor_copy(
                    out=t["hin"][cc][:, b0:b1], in_=t["xnat"][cc][:, b0:b1]
                )

        def _input_q(img, q, t):
            # one input quarter: natural NHWC row loads + bf16 cast.
            _load_blocks(t, 7 * q, 7 * (q + 1))

        def _transp_range(img, rp0, rp1, t):
            # PE-transpose each 2-row block to channel-major and DVE-evict it
            # into the flat padded image (3D strided copy inserts the pad
            # column gap; pad cols/halo stay zero from the one-time memset).
            # On the PE these interleave with conv matmuls of the previous
            # image and never stall it; the DVE evicts depend only on
            # just-produced PSUM, so they cannot block the ob drains behind
            # them on the DVE queue.
            xqh, xql = xqs[img % 4]
            nrp = rp1 - rp0
            for cc in range(2):
                ptq = psb.tile([P, 7, 112], bf16, name="ptb")
                for rpi in range(nrp):
                    nc.tensor.transpose(
                        ptq[:, rpi, :],
                        t["hin"][cc][:, rp0 + rpi, :],
                        identb[:112, :112],
                    )
                # one coarse hi-copy + lo-subtract per (quarter, cc): the
                # 448 fine-grained evicts paid ~0.15us DVE overhead each
                base = 1 + (2 * rp0 + 1) * XR
                ptv = ptq[:, :nrp, :].rearrange(
                    "p rp (two c) -> p rp two c", two=2
                )
                span = 2 * nrp * XR
                dsth = xqh[:, cc, base : base + span].rearrange(
                    "p (rp two c) -> p rp two c", rp=nrp, two=2
                )[:, :, :, :W]
                dstl = xql[:, cc, base : base + span].rearrange(
                    "p (rp two c) -> p rp two c", rp=nrp, two=2
                )[:, :, :, :W]
                nc.vector.tensor_copy(out=dsth, in_=ptv)
                nc.vector.tensor_tensor(
                    out=dstl, in0=ptv, in1=dsth,
                    op=mybir.AluOpType.subtract,
                )

        def _transp_q(img, q, t):
            _transp_range(img, 7 * q, 7 * (q + 1), t)

        def _conv_block(img, bi):
            xqh, xql = xqs[img % 4]
            f0, m, segs = plan[bi]
            ps = psc.tile([P, C], f32, name="ps")
            ci = 0
            # fp8 DoubleRow: one matmul per (tap, hi/lo) contracts BOTH cin
            # chunks -- lhsT [128, 2cc, M] / rhs [128, 2cc, 256] -> [M, 256].
            # x = hi + lo with hi = fp8(x), lo = fp8(x - hi); weights are
            # +-1, exact in fp8e4.
            for src_img in (xqh, xql):
                for ky in range(3):
                    for kx in range(3):
                        s = (ky - 1) * XR + (kx - 1)
                        nc.tensor.matmul(
                            ps[:m, :],
                            lhsT=src_img[:, :, 1 + f0 + s : 1 + f0 + s + m],
                            rhs=wbig[:, 3 * ky + kx],
                            start=(ci == 0),
                            stop=(ci == 17),
                            perf_mode=mybir.MatmulPerfMode.DoubleRow,
                        )
                        ci += 1
            ob = outp.tile([P, C], f32, name="ob")
            nc.vector.tensor_copy(out=ob[:m, :], in_=ps[:m, :])
            for si, (a, e, p0) in enumerate(segs):
                eng = store_engines[(bi + si) % len(store_engines)]
                eng.dma_start(
                    out=y[img, p0 : p0 + (e - a), :], in_=ob[a:e, :]
                )

        INJ_LOAD = (6, 12, 17, 21)  # conv blocks at which the next image's
        INJ_TR = (8, 14, 19, 23)    # quarter loads / PE-transposes are issued
        OWN_Q = {4: 1, 11: 2, 17: 3}  # image 0 stages its own quarter q
        # just before the first conv block that needs those input rows

        def _images():
            t = _alloc_input(0)
            _input_q(0, 0, t)
            wsgn_emit()
            _transp_q(0, 0, t)
            for img in range(ni):
                nxt = _alloc_input(img + 1) if img + 1 < ni else None
                for bi in range(len(plan)):
                    if img == 0 and bi in OWN_Q:
                        _input_q(0, OWN_Q[bi], t)
                        _transp_q(0, OWN_Q[bi], t)
                    if nxt is not None and bi in INJ_LOAD:
                        _input_q(img + 1, INJ_LOAD.index(bi), nxt)
                    if nxt is not None and bi in INJ_TR:
                        _transp_q(img + 1, INJ_TR.index(bi), nxt)
                    _conv_block(img, bi)

        if loops == 1:
            _images()
        else:
            # loop-mode (timing builds): steady-state body with
            # cross-iteration input prefetch -- image (img+1)%ni is staged
            # during conv(img), so iteration boundaries pay no pipeline head.
            # The preamble primes image 0 once; with 4 xq buffers the wrap
            # (conv(3) reading buf 3 while input(0') writes buf 0) is safe.
            t0 = _alloc_input(0)
            _input_q(0, 0, t0)
            wsgn_emit()
            _transp_q(0, 0, t0)
            for q in range(1, 4):
                _input_q(0, q, t0)
                _transp_q(0, q, t0)
            with tc.For_i(0, loops, 1):
                for img in range(ni):
                    nxt_img = (img + 1) % ni
                    nxt = _alloc_input(nxt_img)
                    for bi in range(len(plan)):
                        if bi in INJ_LOAD:
                            _input_q(nxt_img, INJ_LOAD.index(bi), nxt)
                        if bi in INJ_TR:
                            _transp_q(nxt_img, INJ_TR.index(bi), nxt)
                        _conv_block(img, bi)
    nc.compile()
    return nc


def get_bass(ni=NI, loops=1):
    key = (ni, loops)
    if key not in _cache:
        _cache[key] = _build_bass(ni, loops)
    return _cache[key]


def run(inputs, kernel, trace=False, **kw):
    from concourse.bass_utils import run_bass_kernel_spmd

    nc = get_bass()
    xs = np.ascontiguousarray(inputs, dtype=np.float32).reshape(NTOT, NPIX, C)
    wf = np.ascontiguousarray(kernel, dtype=np.float32)
    in_maps = [
        {"x": xs[i * NI : (i + 1) * NI], "w": wf} for i in range(NCORES)
    ]
    res = run_bass_kernel_spmd(nc, in_maps, core_ids=list(range(NCORES)),
                               trace=trace, **kw)
    out = np.concatenate([r["y"] for r in res.results], axis=0)
    return out.reshape(NTOT, H, W, C), res


def kernel(**inputs):
    out, _ = run(inputs["inputs"], inputs["kernel"])
    return out
